# revision 76
# baseline (speedup 1.0000x reference)
"""Trainium2 Bass kernel for a pre-norm transformer block (attention + MLP).

Shapes: x [4, 1024, 1024], H=16 heads, Dh=64, MLP hidden 4096, f32.

Strategy (8 NeuronCores, no collectives):
  - Token-sharded: core c handles batch row b=c//2, query tokens
    [off, off+512), off=(c%2)*512. The host ROTATES each core's row so its
    query window is always columns 0:512 (k-token permutation is invariant
    under softmax+A@V), which removes the separate query-slice input. Both
    cores of a pair redundantly compute K/V over the full 1024-token row
    (cheaper than a pair-exchange collective on this topology).
  - Activations flow in transposed layout [feature(partition), token(free)];
    weights are transposed on the host; every matmul is bf16.
  - All three LayerNorms are folded algebraically into the following matmul's
    eviction: ln(x) @ W'^T = rstd*(x @ W'^T) + (-mu*rstd)*colsum(W'), with
    ln gains folded into W' on the host (LN biases asserted zero).
  - LN stats per token via N=1 matmuls (data chunk as stationary operand,
    ones column as moving operand), one SINGLE-SHOT psum group per
    (source-tile, token-chunk) -- interleaved open accumulation chains
    within one psum bank corrupt results on HW -- then a small DVE tree
    reduction over source tiles. Per-token stat columns feed V directly;
    row forms are recovered with [128,1] transposes against an identity and
    broadcast across partitions with K=1 matmuls.
  - Softmax: S computed with the two 64-row array halves into one 2-bank
    psum tile, a single wide exp per (head-pair, k-tile); denominator folded
    into A@V via an appended ones-column on V. A@V runs transposed (E chunks
    stationary, [V|1] moving, N=65), normalized with per-partition scalar
    reciprocals, and flipped back with paired-head 128x128 PE transposes.
  - Softmax rows for the first two head-pairs are hoisted into the QKV
    phase (right after their K/Q tiles exist) so their ACT exps overlap PE
    matmuls; the A@V loop keeps a deep exp pipeline (E pool, 21 tiles).
  - fc2 streams in three waves: outputs 0-3 accumulate interleaved with fc1
    (lagged one hidden tile behind the gelu evictions), outputs 4-5 stream
    after while the LNh finish + row broadcast overlaps their matmuls, and
    outputs 6-7 reuse the freed psum banks with per-output chains so each
    eviction+store overlaps the next chain; the last output's residual+bias
    sum is precomputed during its chain, leaving a two-op tail.
  - Explicit activation-table loads pin natural_log_exp (Ln/Exp/Square/Copy)
    outside fc1 and gelu_and_others inside it, minimizing table thrash.
"""

import sys

try:
    import concourse  # noqa: F401
except ImportError:  # pragma: no cover
    sys.path.insert(0, "/opt/trn_rl_repo")

import ml_dtypes
import numpy as np

import concourse.bass as bass  # noqa: F401
import concourse.tile as tile
from concourse import bacc, bass_utils, mybir

F32 = mybir.dt.float32
F32R = mybir.dt.float32r
BF16 = mybir.dt.bfloat16
AF = mybir.ActivationFunctionType
OP = mybir.AluOpType

P = 128
C = 1024
N = 1024
B = 4
H = 16
DH = 64
DFF = 4096
TOK = 512          # per-core query-token block
NCT = C // P       # 8 c-tiles
NFT = DFF // P     # 32 d'-tiles
EPS = 1e-5
SCALE = DH ** -0.5

_CACHE = {}


def build():
    nc = bacc.Bacc(
        "TRN2",
        target_bir_lowering=False,
        debug=False,
        enable_asserts=False,
        num_devices=8,
    )

    def din(name, shape, dt=F32R):
        return nc.dram_tensor(name, shape, dt, kind="ExternalInput").ap()

    xrow = din("xrow", [C, N], BF16)      # rotated x[b].T (bf16), q-window first
    wqkv = din("wqkv", [C, 3 * C], BF16)  # (qkv rows * ln1_g).T
    wproj = din("wproj", [C, C], BF16)    # proj_w.T
    wfc1 = din("wfc1", [C, DFF], BF16)    # (fc1_w * ln2_g).T
    wfc2 = din("wfc2", [DFF, C], BF16)    # (fc2_w * lnh_g).T
    wqs = din("wqs", [P, 2 * NCT], F32)   # per-col rowsums of folded q,k weights
    wvs = din("wvs", [1, C], BF16)        # rowsums for V cols (row layout)
    w1s = din("w1s", [P, NFT], F32)       # per-col rowsums of folded fc1
    w2s = din("w2s", [P, NCT], F32)       # per-col rowsums of folded fc2
    pb = din("pb", [P, NCT], F32)
    f1b = din("f1b", [P, NFT], F32)
    f2b = din("f2b", [P, NCT], F32)
    ident_d = din("ident", [P, P], BF16)  # identity (transposes)

    outT = nc.dram_tensor("outT", [C, TOK], F32, kind="ExternalOutput").ap()

    with tile.TileContext(nc) as tc:
        const = tc.alloc_tile_pool(name="const", bufs=1)
        big = tc.alloc_tile_pool(name="big", bufs=1)
        tmp = tc.alloc_tile_pool(name="tmp", bufs=2)
        misc = tc.alloc_tile_pool(name="misc", bufs=1)
        wpool = tc.alloc_tile_pool(name="w", bufs=5)

        # --- load x FIRST (rotated row; query window = cols 0:TOK) — the
        # head of the kernel is DMA-latency-bound, so x outranks constants
        # in the (serialized) DMA dispatch queue.
        xr = big.tile([P, NCT, N], BF16, tag="A")
        xrs = xrow.rearrange("(i p) t -> p i t", p=P)
        wqkv_r = wqkv.rearrange("(i p) n -> p i n", p=P)

        def qkv_wload(g, split=False):
            w = wpool.tile([P, NCT, 512], BF16, tag="w", name=f"wg{g}")
            if split:
                nc.sync.dma_start(w[:, :, 0:256], wqkv_r[:, :, g * 512:g * 512 + 256])
                nc.sync.dma_start(w[:, :, 256:512],
                                  wqkv_r[:, :, g * 512 + 256:(g + 1) * 512])
            else:
                nc.sync.dma_start(w[:], wqkv_r[:, :, g * 512:(g + 1) * 512])
            return w

        wts = {}
        for h_ in range(3):
            nc.sync.dma_start(xr[:, 2 * h_:2 * h_ + 2, :], xrs[:, 2 * h_:2 * h_ + 2, :])
        wts[0] = qkv_wload(0, split=True)
        nc.sync.dma_start(xr[:, 6:8, :], xrs[:, 6:8, :])
        wts[2] = qkv_wload(2, split=True)

        # --- constants ---
        ones1 = const.tile([1, P], BF16)
        nc.vector.memset(ones1[:], 1.0)
        onesc_b = const.tile([P, 1], BF16)
        nc.vector.memset(onesc_b[:], 1.0)
        eps_col = const.tile([P, 1], F32)
        nc.vector.memset(eps_col[:], EPS)
        ident = const.tile([P, P], BF16)
        nc.sync.dma_start(ident[:], ident_d[:])
        wvs_s = const.tile([1, C], BF16)
        nc.sync.dma_start(wvs_s[:], wvs[:])
        gb = {}

        def load_gb():
            for nm, ap_, w in (("wqs", wqs, 2 * NCT), ("pb", pb, NCT),
                               ("f1b", f1b, NFT), ("f2b", f2b, NCT),
                               ("w1s", w1s, NFT), ("w2s", w2s, NCT)):
                t = const.tile([P, w], F32, name=nm, tag=nm)
                nc.sync.dma_start(t[:], ap_[:])
                gb[nm] = t

        def act_table(set_id):
            nc.scalar.add_instruction(mybir.InstLoadActFuncSet(
                name=nc.get_next_instruction_name(), ins=[], outs=[],
                act_func_set_id=set_id))

        act_table(6)   # natural_log_exp_and_others: Ln/Exp/Square/Copy

        # ---------- LN stat helpers (column trick) ----------
        def ln_cols_finish(S, nch, n_elems, name):
            """S: psum [P, 2*nch] (cols 0..nch-1 sums, nch..2nch-1 sumsq).
            Returns (rstd_cb, nmr_cb) [P, nch] bf16 column tiles."""
            inv = 1.0 / n_elems
            mu = misc.tile([P, nch], F32R, tag="lnf", bufs=4, name=f"mu_{name}")
            nc.vector.tensor_scalar_mul(mu[:], S[:, 0:nch], inv)
            ex2 = misc.tile([P, nch], F32, tag="lnf", bufs=4, name=f"ex2_{name}")
            nc.vector.tensor_scalar_mul(ex2[:], S[:, nch:2 * nch], inv)
            mu2 = misc.tile([P, nch], F32, tag="lnf", bufs=4, name=f"mu2_{name}")
            nc.vector.tensor_mul(mu2[:], mu[:], mu[:])
            nc.vector.tensor_sub(ex2[:], ex2[:], mu2[:])
            nc.scalar.activation(ex2[:], ex2[:], AF.Ln, bias=eps_col[:])
            rstd = misc.tile([P, nch], F32, tag="lnc", bufs=6, name=f"rstd_{name}")
            nc.scalar.activation(rstd[:], ex2[:], AF.Exp, scale=-0.5)
            nmr = misc.tile([P, nch], F32, tag="lnc", bufs=6, name=f"nmr_{name}")
            nc.vector.scalar_tensor_tensor(nmr[:], mu[:], -1.0, rstd[:],
                                           op0=OP.mult, op1=OP.mult)
            rstd_cb = misc.tile([P, nch], BF16, tag="lncb", bufs=6, name=f"rcb_{name}")
            nc.vector.tensor_copy(rstd_cb[:], rstd[:])
            nmr_cb = misc.tile([P, nch], BF16, tag="lncb", bufs=6, name=f"ncb_{name}")
            nc.vector.tensor_copy(nmr_cb[:], nmr[:])
            return rstd, nmr, rstd_cb, nmr_cb

        def ln_rows_bcast(ps_row, ps_bc, rstd_cb, nmr_cb, nch, name, act=False):
            """Columns [P, nch] -> broadcast tiles [P, nch*P] bf16 (rstd_b, nmr_b).
            act=True moves the psum->sbuf copies to the Scalar engine so a
            busy DVE stream can't head-of-line-block them."""
            cp = (lambda o, i: nc.scalar.activation(o, i, AF.Copy)) if act \
                else (lambda o, i: nc.vector.tensor_copy(o, i))
            outs = []
            for cb, nm in ((rstd_cb, "r"), (nmr_cb, "n")):
                rowp = ps_row.tile([1, nch * P], BF16, tag="row",
                                   name=f"rp_{name}{nm}")
                for ch in range(nch):
                    nc.tensor.matmul(rowp[0:1, ch * P:(ch + 1) * P],
                                     cb[:, ch:ch + 1], ident[:],
                                     is_transpose=True, start=True, stop=True)
                row = misc.tile([1, nch * P], BF16, tag="lnrow", bufs=2,
                                name=f"row_{name}{nm}")
                cp(row[:], rowp[0:1, :])
                bcast = misc.tile([P, nch * P], BF16, tag=f"lnb{nch}",
                                  bufs=(2 if nch == 8 else 4), name=f"b_{name}{nm}")
                for hh in range(nch * P // TOK):
                    bp = ps_bc.tile([P, TOK], F32, tag="bc", name=f"bc_{name}{nm}{hh}")
                    nc.tensor.matmul(bp[:], ones1[:], row[0:1, hh * TOK:(hh + 1) * TOK],
                                     start=True, stop=True)
                    cp(bcast[:, hh * TOK:(hh + 1) * TOK], bp[:])
                outs.append(bcast)
            return outs

        # --- LN1 stats over the full (rotated) row: 8 chunks of 128 tokens ---
        ps_ln1 = tc.alloc_tile_pool(name="ps_ln1", bufs=1, space="PSUM")
        ps_row = tc.alloc_tile_pool(name="ps_row", bufs=2, space="PSUM")
        ps_bc = tc.alloc_tile_pool(name="ps_bc", bufs=2, space="PSUM")
        # single-shot stat matmuls (one complete psum group per (src, chunk);
        # interleaved open accumulation chains in one bank corrupt results),
        # then a DVE tree-reduction over the source axis.
        SP1 = ps_ln1.tile([P, NCT, 16], F32, tag="SP1")
        for ci in range(NCT):
            # squares split across ACT and DVE to halve serial latency
            s = tmp.tile([P, N], BF16, tag="ev1", bufs=3, name=f"sq1_{ci}")
            if ci % 2 == 0:
                nc.scalar.activation(s[:], xr[:, ci, :], AF.Square)
            else:
                nc.vector.tensor_mul(s[:], xr[:, ci, :], xr[:, ci, :])
            for ch in range(NCT):
                nc.tensor.matmul(SP1[:, ci, ch:ch + 1],
                                 xr[:, ci, ch * P:(ch + 1) * P],
                                 onesc_b[:], start=True, stop=True)
                nc.tensor.matmul(SP1[:, ci, 8 + ch:9 + ch],
                                 s[:, ch * P:(ch + 1) * P],
                                 onesc_b[:], start=True, stop=True)
        sp1c = misc.tile([P, NCT, 16], F32, tag="sp1c", name="sp1c")
        nc.vector.tensor_copy(sp1c[:], SP1[:])
        t1_ = misc.tile([P, 4, 16], F32, tag="tr1a", name="tr1_ln1")
        nc.vector.tensor_add(t1_[:], sp1c[:, 0:4, :], sp1c[:, 4:8, :])
        t2_ = misc.tile([P, 2, 16], F32, tag="tr2a", name="tr2_ln1")
        nc.vector.tensor_add(t2_[:], t1_[:, 0:2, :], t1_[:, 2:4, :])
        S1 = misc.tile([P, 16], F32, tag="tr3a", name="tr3_ln1")
        nc.vector.tensor_add(S1[:], t2_[:, 0, :], t2_[:, 1, :])
        rstd1_c, nmr1_c, rstd1_cb, nmr1_cb = ln_cols_finish(S1, NCT, C, "ln1")
        rstd1_b, nmr1_b = ln_rows_bcast(ps_row, ps_bc, rstd1_cb, nmr1_cb, NCT, "ln1")

        # wvs broadcast to all partitions (for V eviction outer product)
        wvs_b = misc.tile([P, C], BF16, tag="wvs_b")
        for g in range(2):
            bp = ps_bc.tile([P, TOK], F32, tag="bc", name=f"bcv{g}")
            nc.tensor.matmul(bp[:], ones1[:], wvs_s[0:1, g * TOK:(g + 1) * TOK],
                             start=True, stop=True)
            nc.vector.tensor_copy(wvs_b[:, g * TOK:(g + 1) * TOK], bp[:])
        ps_bc.release()
        ps_row.release()
        ps_ln1.release()

        # --- QKV with fused LN1 ---
        KT = big.tile([P, NCT, N], BF16, tag="B")
        QT = big.tile([P, NCT, TOK], BF16, tag="F")
        V = big.tile([P, NCT, H, DH + 1], BF16, tag="V")
        for r in range(NCT):
            nc.vector.memset(V[:, r, :, DH:DH + 1], 1.0)

        ps_s = tc.alloc_tile_pool(name="ps_s", bufs=2, space="PSUM")
        ps_acc = tc.alloc_tile_pool(name="ps_qkv", bufs=4, space="PSUM")

        def qk_group(g, wt):
            if g < 2:  # Q -> QT (query block only)
                for jt in range(4):
                    jj = g * 4 + jt
                    ps = ps_acc.tile([P, TOK], F32, tag="acc")
                    for ci in range(NCT):
                        nc.tensor.matmul(ps[:], wt[:, ci, jt * P:(jt + 1) * P],
                                         xr[:, ci, 0:TOK],
                                         start=(ci == 0), stop=(ci == NCT - 1))
                    t = tmp.tile([P, TOK], BF16, tag="ev", bufs=4)
                    nc.vector.tensor_mul(t[:], ps[:], rstd1_b[:, 0:TOK])
                    nc.vector.scalar_tensor_tensor(
                        QT[:, jj, :], nmr1_b[:, 0:TOK], gb["wqs"][:, jj:jj + 1], t[:],
                        op0=OP.mult, op1=OP.add)
            else:  # K -> KT (full row)
                for jt in range(4):
                    jj = (g - 2) * 4 + jt
                    for blk in range(2):
                        sl = slice(blk * TOK, (blk + 1) * TOK)
                        ps = ps_acc.tile([P, TOK], F32, tag="acc")
                        for ci in range(NCT):
                            nc.tensor.matmul(ps[:], wt[:, ci, jt * P:(jt + 1) * P],
                                             xr[:, ci, sl],
                                             start=(ci == 0), stop=(ci == NCT - 1))
                        t = tmp.tile([P, TOK], BF16, tag="ev", bufs=4)
                        nc.vector.tensor_mul(t[:], ps[:], rstd1_b[:, sl])
                        nc.vector.scalar_tensor_tensor(
                            KT[:, jj, sl], nmr1_b[:, sl],
                            gb["wqs"][:, NCT + jj:NCT + jj + 1], t[:],
                            op0=OP.mult, op1=OP.add)

        def v_group(g, wt):  # g in (4, 5)
            h0 = 8 * (g - 4)
            dsl = slice((g - 4) * TOK, (g - 4 + 1) * TOK)
            for r in range(NCT):
                ps = ps_acc.tile([P, TOK], F32, tag="acc")
                for ci in range(NCT):
                    nc.tensor.matmul(ps[:], xr[:, ci, r * P:(r + 1) * P],
                                     wt[:, ci, :], start=(ci == 0), stop=(ci == NCT - 1))
                t = tmp.tile([P, TOK], BF16, tag="ev", bufs=4)
                nc.vector.tensor_scalar_mul(t[:], wvs_b[:, dsl], nmr1_c[:, r:r + 1])
                nc.vector.scalar_tensor_tensor(
                    V[:, r, h0:h0 + 8, 0:DH],
                    ps[:].rearrange("p (h d) -> p h d", h=8),
                    rstd1_c[:, r:r + 1],
                    t[:].rearrange("p (h d) -> p h d", h=8),
                    op0=OP.mult, op1=OP.add)

        Es = {}

        def s_exp(jj):
            """Hoisted path (PE-bound QKV phase): per-kt psum + per-kt exp."""
            E_l = []
            for kt in range(NCT):
                if kt % 2 == 0:
                    E2 = big.tile([P, 2, 2 * TOK], BF16, tag="E", bufs=10,
                                  name=f"E{jj}_{kt // 2}")
                    E_l.append(E2)
                ks = slice(kt * P, (kt + 1) * P)
                psS = ps_s.tile([P, 2, TOK], F32, tag="S", name=f"S{jj}_{kt}")
                nc.tensor.matmul(psS[:, 0, :], KT[0:64, jj, ks], QT[0:64, jj, :],
                                 start=True, stop=True, tile_position=(0, 0))
                nc.tensor.matmul(psS[:, 1, :], KT[64:128, jj, ks], QT[64:128, jj, :],
                                 start=True, stop=True, tile_position=(64, 0))
                nc.scalar.activation(E_l[kt // 2][:, kt % 2, :], psS[:],
                                     AF.Exp, scale=SCALE)
            Es[jj] = E_l

        def s_exp_m(jj):
            """A@V-loop path (ACT-bound): two k-tiles per psum group, one
            wide exp -- halves the fixed per-op PSUM-access init cost."""
            E_l = []
            for ktp in range(4):
                E2 = big.tile([P, 2, 2 * TOK], BF16, tag="E", bufs=10,
                              name=f"E{jj}_{ktp}")
                E_l.append(E2)
                psS = ps_s2.tile([P, 2, 2, TOK], F32, tag="S4",
                                 name=f"S{jj}_{ktp}")
                for k2 in range(2):
                    kt = 2 * ktp + k2
                    ks = slice(kt * P, (kt + 1) * P)
                    nc.tensor.matmul(psS[:, k2, 0, :], KT[0:64, jj, ks],
                                     QT[0:64, jj, :], start=True, stop=True,
                                     tile_position=(0, 0))
                    nc.tensor.matmul(psS[:, k2, 1, :], KT[64:128, jj, ks],
                                     QT[64:128, jj, :], start=True, stop=True,
                                     tile_position=(64, 0))
                nc.scalar.activation(E2[:], psS[:], AF.Exp, scale=SCALE)
            Es[jj] = E_l

        def av(jj):
            """Transposed A@V: E chunks stationary, [V|1] moving (N=65)."""
            E_l = Es.pop(jj)
            OTT = misc.tile([P, 4, P], BF16, tag="OTT", bufs=4, name=f"OTT{jj}")
            tpj = ps_o.tile([P, 4, P], BF16, tag="tp", bufs=1, name=f"tp{jj}")
            for qc in range(4):
                pO = ps_o.tile([P, 2, DH + 1], F32, tag="O", bufs=2,
                               name=f"pO{jj}_{qc}")
                for h2 in range(2):
                    h = 2 * jj + h2
                    for kt in range(NCT):
                        nc.tensor.matmul(pO[:, h2, :],
                                         E_l[kt // 2][:, kt % 2,
                                                      h2 * TOK + qc * P:
                                                      h2 * TOK + (qc + 1) * P],
                                         V[:, kt, h, :],
                                         start=(kt == 0), stop=(kt == NCT - 1))
                rec = misc.tile([P, 2], F32, tag="rec", bufs=3, name=f"rec{jj}_{qc}")
                for h2 in range(2):
                    with nc.allow_low_precision(reason="softmax denom"):
                        nc.vector.reciprocal(rec[:, h2:h2 + 1], pO[:, h2, DH:DH + 1])
                    nc.vector.tensor_scalar_mul(
                        OTT[:, qc, h2 * DH:(h2 + 1) * DH], pO[:, h2, 0:DH],
                        rec[:, h2:h2 + 1])
                nc.tensor.matmul(tpj[:, qc, :], OTT[:, qc, :], ident[:],
                                 is_transpose=True, start=True, stop=True)
            nc.vector.tensor_copy(OT[:, jj, :], tpj[:])

        OT = big.tile([P, NCT, TOK], BF16, tag="C")

        load_gb()
        wts[1] = qkv_wload(1)
        qk_group(0, wts.pop(0))
        wts[3] = qkv_wload(3)
        qk_group(2, wts.pop(2))
        wts[4] = qkv_wload(4)
        # softmax rows for jj 0,1 as soon as their K/Q tiles exist: the ACT
        # exp stream starts ~7us earlier and overlaps the remaining QKV
        # matmuls, so the A@V phase is less exp-starved
        s_exp(0)
        s_exp(1)
        wts[5] = qkv_wload(5)
        qk_group(1, wts.pop(1))
        qk_group(3, wts.pop(3))
        v_group(4, wts.pop(4))
        v_group(5, wts.pop(5))
        ps_acc.release()
        ps_o = tc.alloc_tile_pool(name="ps_o", bufs=2, space="PSUM")
        for jj in range(NCT):
            av(jj)
            if jj + 2 < NCT:
                s_exp(jj + 2)
        ps_o.release()
        ps_s.release()

        # --- output projection + residual -> x2 (f32r) & x2b (bf16) ---
        x2 = big.tile([P, NCT, TOK], F32R, tag="D")
        x2b = big.tile([P, NCT, TOK], BF16, tag="F")  # reuses QT slot
        ps_ln2 = tc.alloc_tile_pool(name="ps_ln2", bufs=1, space="PSUM")
        SP2 = ps_ln2.tile([P, NCT, 8], F32, tag="SP2")
        ps_acc = tc.alloc_tile_pool(name="ps_proj", bufs=4, space="PSUM")
        wproj_r = wproj.rearrange("(i p) n -> p i n", p=P)
        for ig in range(2):
            w = wpool.tile([P, NCT, 512], BF16, tag="w", name=f"wp{ig}")
            nc.sync.dma_start(w[:], wproj_r[:, :, ig * 512:(ig + 1) * 512])
            for i4 in range(4):
                i = ig * 4 + i4
                ps = ps_acc.tile([P, TOK], F32, tag="acc")
                for ci in range(NCT):
                    nc.tensor.matmul(ps[:], w[:, ci, i4 * P:(i4 + 1) * P],
                                     OT[:, ci, :], start=(ci == 0), stop=(ci == NCT - 1))
                nc.vector.scalar_tensor_tensor(
                    x2[:, i, :], ps[:], gb["pb"][:, i:i + 1], xr[:, i, 0:TOK],
                    op0=OP.add, op1=OP.add)
                nc.scalar.activation(x2b[:, i, :], x2[:, i, :], AF.Copy)
                sq = tmp.tile([P, TOK], BF16, tag="ln_sq")
                nc.vector.tensor_mul(sq[:], x2[:, i, :], x2[:, i, :])
                for ch in range(4):
                    nc.tensor.matmul(SP2[:, i, ch:ch + 1],
                                     x2b[:, i, ch * P:(ch + 1) * P], onesc_b[:],
                                     start=True, stop=True)
                    nc.tensor.matmul(SP2[:, i, 4 + ch:5 + ch],
                                     sq[:, ch * P:(ch + 1) * P], onesc_b[:],
                                     start=True, stop=True)
        ps_acc.release()

        sp2c = misc.tile([P, NCT, 8], F32, tag="sp2c", name="sp2c")
        nc.vector.tensor_copy(sp2c[:], SP2[:])
        t1_ = misc.tile([P, 4, 8], F32, tag="tr1b", name="tr1_ln2")
        nc.vector.tensor_add(t1_[:], sp2c[:, 0:4, :], sp2c[:, 4:8, :])
        t2_ = misc.tile([P, 2, 8], F32, tag="tr2b", name="tr2_ln2")
        nc.vector.tensor_add(t2_[:], t1_[:, 0:2, :], t1_[:, 2:4, :])
        S2 = misc.tile([P, 8], F32, tag="tr3b", name="tr3_ln2")
        nc.vector.tensor_add(S2[:], t2_[:, 0, :], t2_[:, 1, :])
        _, _, rstd2_cb, nmr2_cb = ln_cols_finish(S2, 4, C, "ln2")
        ps_row = tc.alloc_tile_pool(name="ps_row2", bufs=2, space="PSUM")
        ps_bc = tc.alloc_tile_pool(name="ps_bc2", bufs=2, space="PSUM")
        rstd2_b, nmr2_b = ln_rows_bcast(ps_row, ps_bc, rstd2_cb, nmr2_cb, 4, "ln2", act=True)
        ps_bc.release()
        ps_row.release()
        ps_ln2.release()

        # --- fc1 (LN2 folded into eviction) + LNh stats + fc2 first half ---
        U0 = big.tile([P, NFT // 2, TOK], BF16, tag="B")   # reuses KT slot
        U1 = big.tile([P, NFT // 2, TOK], BF16, tag="V")   # reuses V slot

        def u_tile(i):
            return (U0 if i < NFT // 2 else U1)[:, i % (NFT // 2), :]

        ps_f2 = tc.alloc_tile_pool(name="ps_fc2", bufs=1, space="PSUM")
        fp2 = [ps_f2.tile([P, TOK], F32, tag=f"f2_{j}", name=f"f2_{j}")
               for j in range(4)]
        ps_lnh = tc.alloc_tile_pool(name="ps_lnh", bufs=1, space="PSUM")
        SPh = ps_lnh.tile([P, NFT, 8], F32, tag="SPh")
        ps_f1 = tc.alloc_tile_pool(name="ps_fc1", bufs=3, space="PSUM")
        wfc1_r = wfc1.rearrange("(i p) n -> p i n", p=P)
        wfc2_r = wfc2.rearrange("(i p) n -> p i n", p=P)
        w1t = {}
        w2t = {}
        sqh = {}

        def fc1_step(i):
            ig, i4 = divmod(i, 4)
            if i4 == 0:
                w = wpool.tile([P, NCT, 512], BF16, tag="w", name=f"w1_{ig}")
                nc.sync.dma_start(w[:], wfc1_r[:, :, ig * 512:(ig + 1) * 512])
                w1t[ig] = w
                if ig > 0:
                    del w1t[ig - 1]
            if i % 8 == 0:
                cc = i // 8
                w = wpool.tile([P, NCT, 512], BF16, tag="w", name=f"w2a_{cc}")
                nc.sync.dma_start(w[:], wfc2_r[:, cc * 8:cc * 8 + 8, 0:512])
                w2t[cc] = w
            w = w1t[i // 4]
            ps = ps_f1.tile([P, TOK], F32, tag="acc")
            for ci in range(NCT):
                nc.tensor.matmul(ps[:], w[:, ci, (i % 4) * P:(i % 4 + 1) * P],
                                 x2b[:, ci, :], start=(ci == 0), stop=(ci == NCT - 1))
            t = tmp.tile([P, TOK], F32R, tag="ev1", bufs=3)
            nc.vector.tensor_mul(t[:], ps[:], rstd2_b[:])
            t2 = tmp.tile([P, TOK], F32R, tag="ev2", bufs=3)
            nc.vector.scalar_tensor_tensor(t2[:], nmr2_b[:], gb["w1s"][:, i:i + 1],
                                           t[:], op0=OP.mult, op1=OP.add)
            nc.scalar.activation(u_tile(i), t2[:], AF.Gelu, bias=gb["f1b"][:, i:i + 1])
            s = tmp.tile([P, TOK], BF16, tag="sqh", bufs=3, name=f"sqh{i}")
            nc.vector.tensor_mul(s[:], u_tile(i), u_tile(i))
            sqh[i] = s

        def hstats_and_fc2a(i):
            s = sqh.pop(i)
            for ch in range(4):
                nc.tensor.matmul(SPh[:, i, ch:ch + 1],
                                 u_tile(i)[:, ch * P:(ch + 1) * P], onesc_b[:],
                                 start=True, stop=True)
                nc.tensor.matmul(SPh[:, i, 4 + ch:5 + ch],
                                 s[:, ch * P:(ch + 1) * P], onesc_b[:],
                                 start=True, stop=True)
            w = w2t[i // 8]
            for j in range(4):
                nc.tensor.matmul(fp2[j][:], w[:, i % 8, j * P:(j + 1) * P],
                                 u_tile(i), start=(i == 0), stop=(i == NFT - 1))

        act_table(10)  # gelu_and_others: Gelu/Square/Copy
        for i in range(NFT):
            fc1_step(i)
            if i >= 1:
                hstats_and_fc2a(i - 1)
        hstats_and_fc2a(NFT - 1)
        ps_f1.release()
        act_table(6)   # back to Ln/Exp for the LNh finish

        sphc = misc.tile([P, NFT, 8], F32, tag="sphc", name="sphc")
        nc.vector.tensor_copy(sphc[:], SPh[:])
        th1 = misc.tile([P, 16, 8], F32, tag="treeh1", name="trh1")
        nc.vector.tensor_add(th1[:], sphc[:, 0:16, :], sphc[:, 16:32, :])
        th2 = misc.tile([P, 8, 8], F32, tag="treeh2", name="trh2")
        nc.vector.tensor_add(th2[:], th1[:, 0:8, :], th1[:, 8:16, :])
        th3 = misc.tile([P, 4, 8], F32, tag="tr1b", name="trh3")
        nc.vector.tensor_add(th3[:], th2[:, 0:4, :], th2[:, 4:8, :])
        th4 = misc.tile([P, 2, 8], F32, tag="tr2b", name="trh4")
        nc.vector.tensor_add(th4[:], th3[:, 0:2, :], th3[:, 2:4, :])
        Sh = misc.tile([P, 8], F32, tag="tr3b", name="trh5")
        nc.vector.tensor_add(Sh[:], th4[:, 0, :], th4[:, 1, :])
        _, _, rstdh_cb, nmrh_cb = ln_cols_finish(Sh, 4, DFF, "lnh")
        ps_lnh.release()

        OS = big.tile([P, NCT, TOK], F32, tag="A")  # reuses xr slot

        def fc2_evict(j, fps):
            t = tmp.tile([P, TOK], F32R, tag="ev1", bufs=3)
            nc.vector.tensor_mul(t[:], fps[:], rstdh_b[:])
            t2 = tmp.tile([P, TOK], F32R, tag="ev2", bufs=3)
            nc.vector.scalar_tensor_tensor(t2[:], nmrh_b[:], gb["w2s"][:, j:j + 1],
                                           t[:], op0=OP.mult, op1=OP.add)
            nc.vector.scalar_tensor_tensor(OS[:, j, :], t2[:], gb["f2b"][:, j:j + 1],
                                           x2[:, j, :], op0=OP.add, op1=OP.add)

        # --- fc2 second half streams; first-half evictions overlap it.
        # The LNh row-broadcast is emitted a few iterations into pass 2a so
        # its PE transposes queue behind already-runnable matmuls while the
        # ACT finish chain (table load + Ln/Exp) completes.
        ps_f2b = tc.alloc_tile_pool(name="ps_fc2b", bufs=1, space="PSUM")
        fp2b = [ps_f2b.tile([P, TOK], F32, tag=f"f2b_{j}", name=f"f2b_{j}")
                for j in range(2)]
        outT_r = outT.rearrange("(i p) t -> p i t", p=P)
        rstdh_b = nmrh_b = None
        # pass 2a: outputs j=4,5; loads the second-half fc2 weight chunks
        for i in range(NFT):
            if i % 8 == 0:
                cc = i // 8
                w = wpool.tile([P, NCT, 512], BF16, tag="w", name=f"w2b_{cc}")
                nc.sync.dma_start(w[:], wfc2_r[:, cc * 8:cc * 8 + 8, 512:1024])
                w2t[4 + cc] = w
            w = w2t[4 + i // 8]
            for j in range(2):
                nc.tensor.matmul(fp2b[j][:], w[:, i % 8, j * P:(j + 1) * P],
                                 u_tile(i), start=(i == 0), stop=(i == NFT - 1))
            if i == 7:
                ps_row = tc.alloc_tile_pool(name="ps_rowh", bufs=1, space="PSUM")
                ps_bc = tc.alloc_tile_pool(name="ps_bch", bufs=1, space="PSUM")
                rstdh_b, nmrh_b = ln_rows_bcast(ps_row, ps_bc,
                                                rstdh_cb, nmrh_cb, 4, "lnh",
                                                act=True)
                ps_bc.release()
                ps_row.release()
            if i == 12:
                for j in range(4):
                    fc2_evict(j, fp2[j])
                nc.sync.dma_start(outT_r[:, 0:4, :], OS[:, 0:4, :])
        fc2_evict(4, fp2b[0])
        fc2_evict(5, fp2b[1])
        nc.sync.dma_start(outT_r[:, 4:6, :], OS[:, 4:6, :])
        # pass 2b: outputs j=6,7 re-use the two psum banks (their previous
        # accumulators were just evicted) and the still-resident weight chunks
        fp2c = [ps_f2b.tile([P, TOK], F32, tag=f"f2b_{j}", name=f"f2c_{j}")
                for j in range(2)]
        for j in range(2):
            for i in range(NFT):
                w = w2t[4 + i // 8]
                nc.tensor.matmul(fp2c[j][:],
                                 w[:, i % 8, (2 + j) * P:(3 + j) * P],
                                 u_tile(i), start=(i == 0), stop=(i == NFT - 1))
            if j == 0:
                fc2_evict(6, fp2c[0])
                nc.sync.dma_start(outT_r[:, 6:7, :], OS[:, 6:7, :])
                # precompute the j=7 residual+bias sum while j=7's chain
                # streams, shortening the post-chain eviction to two ops
                nw7 = tmp.tile([P, TOK], F32R, tag="ev2", bufs=3, name="nw7")
                nc.vector.tensor_scalar(nw7[:], nmrh_b[:], gb["w2s"][:, 7:8],
                                        gb["f2b"][:, 7:8], op0=OP.mult,
                                        op1=OP.add)
                pre7 = tmp.tile([P, TOK], F32R, tag="ev1", bufs=3, name="pre7")
                nc.vector.tensor_add(pre7[:], nw7[:], x2[:, 7, :])
        t7 = tmp.tile([P, TOK], F32R, tag="ev2", bufs=3, name="t7")
        nc.vector.tensor_mul(t7[:], fp2c[1][:], rstdh_b[:])
        nc.vector.tensor_add(OS[:, 7, :], t7[:], pre7[:])
        nc.sync.dma_start(outT_r[:, 7:8, :], OS[:, 7:8, :])
        ps_f2b.release()
        ps_f2.release()

        for p_ in (wpool, misc, tmp, big, const):
            p_.release()

    nc.compile()
    return nc


def _prep_inputs(inputs):
    """Host-side transposes/folds/rotations -> per-core in_maps."""
    f = lambda a: np.asarray(a, dtype=np.float32)
    x = f(inputs["x"])
    xT = np.ascontiguousarray(x.transpose(0, 2, 1))          # [B, C, N]

    g1 = f(inputs["ln1_g"])
    g2 = f(inputs["ln2_g"])
    ghv = f(inputs["lnh_g"])
    for nm in ("ln1_b", "ln2_b", "lnh_b"):
        if np.abs(f(inputs[nm])).max() != 0.0:
            raise NotImplementedError(f"{nm} != 0 not supported by this kernel")

    qkv_f = f(inputs["qkv_w"]) * g1[None, :]      # fold ln1_g
    fc1_f = f(inputs["fc1_w"]) * g2[None, :]      # fold ln2_g
    fc2_f = f(inputs["fc2_w"]) * ghv[None, :]     # fold lnh_g
    qs = qkv_f.sum(axis=1)                        # [3072] rowsums

    bf = ml_dtypes.bfloat16
    common = {
        "wqkv": np.ascontiguousarray(qkv_f.T.astype(bf)),
        "wproj": np.ascontiguousarray(f(inputs["proj_w"]).T.astype(bf)),
        "wfc1": np.ascontiguousarray(fc1_f.T.astype(bf)),
        "wfc2": np.ascontiguousarray(fc2_f.T.astype(bf)),
        "wqs": np.ascontiguousarray(qs[:2 * C].reshape(2 * NCT, P).T),
        "wvs": np.ascontiguousarray(qs[2 * C:].reshape(1, C).astype(bf)),
        "w1s": np.ascontiguousarray(fc1_f.sum(axis=1).reshape(NFT, P).T),
        "w2s": np.ascontiguousarray(fc2_f.sum(axis=1).reshape(NCT, P).T),
        "pb": np.ascontiguousarray(f(inputs["proj_b"]).reshape(NCT, P).T),
        "f1b": np.ascontiguousarray(f(inputs["fc1_b"]).reshape(NFT, P).T),
        "f2b": np.ascontiguousarray(f(inputs["fc2_b"]).reshape(NCT, P).T),
        "ident": np.eye(P, dtype=bf),
    }
    in_maps = []
    for c in range(8):
        b, off = c // 2, (c % 2) * TOK
        m = dict(common)
        xb = xT[b].astype(bf)
        m["xrow"] = np.ascontiguousarray(
            np.concatenate([xb[:, off:off + TOK], xb[:, TOK - off:N - off]], axis=1))
        in_maps.append(m)
    return in_maps


def _assemble(results):
    out = np.empty((B, N, C), np.float32)
    for c in range(8):
        b, off = c // 2, (c % 2) * TOK
        out[b, off:off + TOK, :] = results[c]["outT"].T
    return out


def kernel(**inputs) -> np.ndarray:
    nc = _CACHE.get("nc")
    if nc is None:
        nc = build()
        _CACHE["nc"] = nc
    in_maps = _prep_inputs(inputs)
    res = bass_utils.run_bass_kernel_spmd(nc, in_maps, core_ids=list(range(8)))
    return _assemble(res.results)
